# revision 30
# baseline (speedup 1.0000x reference)
"""Dense multi-head attention (B=4, H=16, L=2048, D=64, fp32) on 8 trn2 cores.

Sharding: the 64 (batch, head) pairs split 8-per-core (core c gets batch c//2,
heads (c%2)*8 .. +8); each core computes full attention for its heads with no
cross-core communication. The host pre-transposes Q/K to d-major and appends a
ones column to V while staging per-core inputs (fp16 — S and O still
accumulate in fp32 on-chip, so output error stays ~1e-3-scale-relative).

Per-core kernel (per head, ACT-exp bound at ~33.5M exp elements/core):
  - Q^T, K^T staged d-major in SBUF ([128, 2048] with the 64 d-rows duplicated
    in both partition halves so two K-tiles of the D=64-contraction QK matmul
    run concurrently via tile_position row-packing).
  - S^T tiles [128 k, 512 q] = K^T.T @ Q^T in fp32 PSUM ([128, 1024] groups,
    triple-buffered: 6 of 8 PSUM banks).
  - ACT computes P^T = exp(S^T / 8) PSUM->SBUF (immediate-value bias: skips
    the per-call const-AP bias read bass would otherwise emit).
  - O^T_ext [65, 512] accumulates V_ext.T @ P^T in PSUM over the 16 k-tiles,
    where V_ext = [V | ones] so row 64 is the softmax denominator.
  - Epilogue (software-pipelined one (h, q) behind compute): PE transposes
    [65, 128] chunks back to row-major, DVE reciprocal of the denominators,
    per-partition tensor_scalar multiply normalizes, DMA out row-major.
"""

import numpy as np

import concourse.bass as bass
import concourse.mybir as mybir
import concourse.tile as tile
from concourse import bass_utils
from concourse.masks import make_identity

B, H, L, D = 4, 16, 2048, 64
N_CORES = 8
HEADS_PER_CORE = (B * H) // N_CORES  # 8
KT = L // 128  # 16 k-tiles per head
QT = L // 512  # 4 q-tiles per head
GROUP = 2  # k-tiles per ACT exp call ([128, 1024] PSUM group)
SCALE = 1.0 / float(np.sqrt(D))

F32 = mybir.dt.float32
F32R = mybir.dt.float32r
F16 = mybir.dt.float16
MM_DTYPE = F16  # matmul input dtype: F16 (1 cyc/row) or F32R (2 cyc/row)


def _split_sync_waits(nc):
    """This container's walrus build rejects instructions carrying more than
    one sem wait ("Too many sync wait commands" in setupSyncWait). Splitting
    is semantics-preserving: a same-engine NoOp carrying one of the waits is
    spliced in front, and the sequencer blocks on each in order."""
    for f in nc.m.functions:
        for bb in f.blocks:
            insts = bb.instructions
            out = []
            changed = False
            for inst in insts:
                si = inst.sync_info
                if si is not None and si.on_wait and len(si.on_wait) > 1:
                    waits = list(si.on_wait)
                    for j, w in enumerate(waits[:-1]):
                        nop = mybir.InstNoOp(
                            name=f"{inst.name}_sw{j}",
                            engine=inst.engine,
                            sync_info=mybir.SyncInfo(on_wait=[w], on_update=[]),
                        )
                        out.append(nop)
                    si.on_wait = [waits[-1]]
                    changed = True
                out.append(inst)
            if changed:
                insts[:] = out


def _act_exp_imm(nc, out, in_, scale):
    """ACTIVATE Exp with immediate (non-AP) bias, skipping the const-AP
    conversion bass applies for non-Copy funcs (saves a per-call SBUF
    bias read)."""
    eng = nc.scalar
    inputs = [
        eng.lower_ap(in_),
        mybir.ImmediateValue(dtype=mybir.dt.float32, value=0.0),
        mybir.ImmediateValue(dtype=mybir.dt.float32, value=float(scale)),
        mybir.ImmediateValue(dtype=mybir.dt.float32, value=0.0),
    ]
    outputs = [eng.lower_ap(out)]
    return eng.add_instruction(
        mybir.InstActivation(
            name=nc.get_next_instruction_name(),
            func=mybir.ActivationFunctionType.Exp,
            ins=inputs,
            outs=outputs,
        )
    )


def build_nc(mm_dtype=MM_DTYPE):
    nc = bass.Bass("TRN2", target_bir_lowering=False, debug=False)

    MD = mm_dtype
    qt_d = nc.dram_tensor("qt", [HEADS_PER_CORE, 2 * D, L], MD, kind="ExternalInput")
    kt_d = nc.dram_tensor("kt", [HEADS_PER_CORE, 2 * D, L], MD, kind="ExternalInput")
    v_d = nc.dram_tensor("v", [HEADS_PER_CORE, L, D + 1], MD, kind="ExternalInput")
    o_d = nc.dram_tensor("o", [HEADS_PER_CORE, L, D], F32, kind="ExternalOutput")

    with tile.TileContext(nc) as tc:
        with (
            tc.tile_pool(name="consts", bufs=1) as consts,
            tc.tile_pool(name="qk", bufs=2) as qk_pool,
            tc.tile_pool(name="vx", bufs=2) as vx_pool,
            tc.tile_pool(name="pt", bufs=6) as pt_pool,
            tc.tile_pool(name="osb", bufs=3) as osb_pool,
            tc.tile_pool(name="small", bufs=3) as small_pool,
            tc.tile_pool(name="outsb", bufs=3) as outsb_pool,
            tc.tile_pool(name="st", bufs=3, space="PSUM") as st_pool,
            tc.tile_pool(name="otp", bufs=1, space="PSUM") as ot_pool,
            tc.tile_pool(name="ops", bufs=1, space="PSUM") as ops_pool,
        ):
            identity = consts.tile([128, 128], F32)
            make_identity(nc, identity)
            # Dummy activation so walrus's ACT table load (~2.7us) runs
            # during the first input DMAs instead of before the first real
            # exp call.
            warm = consts.tile([1, 8], F32)
            nc.vector.memset(warm[:], 0.0)
            nc.scalar.activation(warm[:], warm[:], mybir.ActivationFunctionType.Exp)

            def emit_epilogue(h, q, ot):
                """Transpose O^T_ext back to row-major and normalize by the
                softmax denominators (row 64), then DMA out."""
                osb = osb_pool.tile([D + 1, 512], F32)
                nc.vector.tensor_copy(osb[:], ot[:])
                ops = ops_pool.tile([128, 4, D + 1], F32)
                for qq in range(4):
                    nc.tensor.transpose(
                        ops[:, qq, :],
                        osb[:, qq * 128 : (qq + 1) * 128],
                        identity[0 : D + 1, 0 : D + 1],
                    )
                rcp = small_pool.tile([128, 4], F32)
                nc.vector.reciprocal(rcp[:], ops[:, :, D])
                outsb = outsb_pool.tile([128, 4, D], F32)
                for qq in range(4):
                    nc.vector.tensor_scalar_mul(
                        outsb[:, qq, :], ops[:, qq, 0:D], rcp[:, qq : qq + 1]
                    )
                nc.sync.dma_start(
                    o_d.ap()[h, q * 512 : (q + 1) * 512, :].rearrange(
                        "(a p) d -> p a d", p=128
                    ),
                    outsb[:],
                )

            pending = None  # deferred epilogue of the previous (h, q),
            # emitted after the next tile's first QK group so the PE FIFO
            # isn't head-of-line blocked on its dependencies.
            for h in range(HEADS_PER_CORE):
                qt2 = qk_pool.tile([128, L], MD, tag="qt")
                kt2 = qk_pool.tile([128, L], MD, tag="kt")
                for lo in (0, 1):
                    sl = slice(lo * (L // 2), (lo + 1) * (L // 2))
                    nc.sync.dma_start(qt2[:, sl], qt_d.ap()[h][:, sl])
                    nc.sync.dma_start(kt2[:, sl], kt_d.ap()[h][:, sl])
                vx = vx_pool.tile([128, KT, D + 1], MD)
                v_r = v_d.ap()[h].rearrange("(t p) d -> p t d", p=128)
                for c in range(4):
                    nc.sync.dma_start(
                        vx[:, c * 4 : (c + 1) * 4, :], v_r[:, c * 4 : (c + 1) * 4, :]
                    )

                for q in range(QT):
                    ot = ot_pool.tile([D + 1, 512], F32)
                    for g in range(KT // GROUP):
                        st = st_pool.tile([128, 512 * GROUP], F32, tag="st")
                        for i in range(GROUP):
                            kt_idx = GROUP * g + i
                            half = 64 * (kt_idx % 2)
                            nc.tensor.matmul(
                                st[:, i * 512 : (i + 1) * 512],
                                lhsT=kt2[half : half + 64, kt_idx * 128 : (kt_idx + 1) * 128],
                                rhs=qt2[half : half + 64, q * 512 : (q + 1) * 512],
                                start=True,
                                stop=True,
                                tile_position=(half, 0),
                            )
                        pt = pt_pool.tile([128, 512 * GROUP], MD)
                        _act_exp_imm(nc, pt[:], st[:], SCALE)
                        if g == 0 and pending is not None:
                            emit_epilogue(*pending)
                            pending = None
                        for i in range(GROUP):
                            kt_idx = GROUP * g + i
                            nc.tensor.matmul(
                                ot[:, :],
                                lhsT=vx[:, kt_idx, :],
                                rhs=pt[:, i * 512 : (i + 1) * 512],
                                start=(kt_idx == 0),
                                stop=(kt_idx == KT - 1),
                                skip_group_check=True,
                            )
                    pending = (h, q, ot)
            emit_epilogue(*pending)
    _split_sync_waits(nc)
    return nc


def shard_inputs(query, key, value, mm_dtype=MM_DTYPE):
    """Full [B, H, L, D] inputs -> per-core input maps (host-side layout)."""
    np_dt = mybir.dt.np(mm_dtype)
    q = np.asarray(query, dtype=np.float32).reshape(B * H, L, D).astype(np_dt)
    k = np.asarray(key, dtype=np.float32).reshape(B * H, L, D).astype(np_dt)
    v = np.asarray(value, dtype=np.float32).reshape(B * H, L, D).astype(np_dt)
    ones = np.ones((HEADS_PER_CORE, L, 1), np_dt)
    in_maps = []
    for c in range(N_CORES):
        sl = slice(c * HEADS_PER_CORE, (c + 1) * HEADS_PER_CORE)
        in_maps.append(
            {
                "qt": np.ascontiguousarray(
                    np.concatenate([q[sl].transpose(0, 2, 1)] * 2, axis=1)
                ),
                "kt": np.ascontiguousarray(
                    np.concatenate([k[sl].transpose(0, 2, 1)] * 2, axis=1)
                ),
                "v": np.ascontiguousarray(np.concatenate([v[sl], ones], axis=-1)),
            }
        )
    return in_maps


def unshard(results):
    """Per-core [heads, L, D] outputs -> full [B, L, H*D]."""
    o = np.concatenate([r["o"] for r in results], axis=0)  # [B*H, L, D]
    o = o.reshape(B, H, L, D).transpose(0, 2, 1, 3).reshape(B, L, H * D)
    return np.ascontiguousarray(o)


_NC_CACHE = {}


def run(query, key, value, trace=False, mm_dtype=MM_DTYPE):
    key_ = mm_dtype
    if key_ not in _NC_CACHE:
        _NC_CACHE[key_] = build_nc(mm_dtype)
    nc = _NC_CACHE[key_]
    in_maps = shard_inputs(query, key, value, mm_dtype)
    res = bass_utils.run_bass_kernel_spmd(
        nc, in_maps, core_ids=list(range(N_CORES)), trace=trace
    )
    return unshard(res.results), res


def kernel(query, key, value, mask=None, to_q=None, to_k=None):
    out, _ = run(query, key, value, trace=False)
    return out


if __name__ == "__main__":
    rng = np.random.default_rng(0)
    q = rng.normal(size=(B, H, L, D)).astype(np.float32)
    k = rng.normal(size=(B, H, L, D)).astype(np.float32)
    v = rng.normal(size=(B, H, L, D)).astype(np.float32)
    out = kernel(q, k, v)
    print("out", out.shape, out.dtype)


# revision 31
# speedup vs baseline: 1.0070x; 1.0070x over previous
"""Dense multi-head attention (B=4, H=16, L=2048, D=64, fp32) on 8 trn2 cores.

Sharding: the 64 (batch, head) pairs split 8-per-core (core c gets batch c//2,
heads (c%2)*8 .. +8); each core computes full attention for its heads with no
cross-core communication. The host pre-transposes Q/K to d-major and appends a
ones column to V while staging per-core inputs (fp16 — S and O still
accumulate in fp32 on-chip, so output error stays ~1e-3-scale-relative).

Per-core kernel (per head, ACT-exp bound at ~33.5M exp elements/core):
  - Q^T, K^T staged d-major in SBUF ([128, 2048] with the 64 d-rows duplicated
    in both partition halves so two K-tiles of the D=64-contraction QK matmul
    run concurrently via tile_position row-packing).
  - S^T tiles [128 k, 512 q] = K^T.T @ Q^T in fp32 PSUM ([128, 1024] groups,
    triple-buffered: 6 of 8 PSUM banks).
  - ACT computes P^T = exp(S^T / 8) PSUM->SBUF (immediate-value bias: skips
    the per-call const-AP bias read bass would otherwise emit).
  - O^T_ext [65, 512] accumulates V_ext.T @ P^T in PSUM over the 16 k-tiles,
    where V_ext = [V | ones] so row 64 is the softmax denominator.
  - Epilogue (software-pipelined one (h, q) behind compute): PE transposes
    [65, 128] chunks back to row-major, DVE reciprocal of the denominators,
    per-partition tensor_scalar multiply normalizes, DMA out row-major.
"""

import numpy as np

import concourse.bass as bass
import concourse.mybir as mybir
import concourse.tile as tile
from concourse import bass_utils
from concourse.masks import make_identity

B, H, L, D = 4, 16, 2048, 64
N_CORES = 8
HEADS_PER_CORE = (B * H) // N_CORES  # 8
KT = L // 128  # 16 k-tiles per head
QT = L // 512  # 4 q-tiles per head
GROUP = 2  # k-tiles per ACT exp call ([128, 1024] PSUM group)
SCALE = 1.0 / float(np.sqrt(D))

F32 = mybir.dt.float32
F32R = mybir.dt.float32r
F16 = mybir.dt.float16
MM_DTYPE = F16  # matmul input dtype: F16 (1 cyc/row) or F32R (2 cyc/row)


def _split_sync_waits(nc):
    """This container's walrus build rejects instructions carrying more than
    one sem wait ("Too many sync wait commands" in setupSyncWait). Splitting
    is semantics-preserving: a same-engine NoOp carrying one of the waits is
    spliced in front, and the sequencer blocks on each in order."""
    for f in nc.m.functions:
        for bb in f.blocks:
            insts = bb.instructions
            out = []
            changed = False
            for inst in insts:
                si = inst.sync_info
                if si is not None and si.on_wait and len(si.on_wait) > 1:
                    waits = list(si.on_wait)
                    for j, w in enumerate(waits[:-1]):
                        nop = mybir.InstNoOp(
                            name=f"{inst.name}_sw{j}",
                            engine=inst.engine,
                            sync_info=mybir.SyncInfo(on_wait=[w], on_update=[]),
                        )
                        out.append(nop)
                    si.on_wait = [waits[-1]]
                    changed = True
                out.append(inst)
            if changed:
                insts[:] = out


def _act_exp_imm(nc, out, in_, scale):
    """ACTIVATE Exp with immediate (non-AP) bias, skipping the const-AP
    conversion bass applies for non-Copy funcs (saves a per-call SBUF
    bias read)."""
    eng = nc.scalar
    inputs = [
        eng.lower_ap(in_),
        mybir.ImmediateValue(dtype=mybir.dt.float32, value=0.0),
        mybir.ImmediateValue(dtype=mybir.dt.float32, value=float(scale)),
        mybir.ImmediateValue(dtype=mybir.dt.float32, value=0.0),
    ]
    outputs = [eng.lower_ap(out)]
    return eng.add_instruction(
        mybir.InstActivation(
            name=nc.get_next_instruction_name(),
            func=mybir.ActivationFunctionType.Exp,
            ins=inputs,
            outs=outputs,
        )
    )


def build_nc(mm_dtype=MM_DTYPE):
    nc = bass.Bass("TRN2", target_bir_lowering=False, debug=False)

    MD = mm_dtype
    qt_d = nc.dram_tensor("qt", [HEADS_PER_CORE, D, L], MD, kind="ExternalInput")
    kt_d = nc.dram_tensor("kt", [HEADS_PER_CORE, D, L], MD, kind="ExternalInput")
    v_d = nc.dram_tensor("v", [HEADS_PER_CORE, L, D + 1], MD, kind="ExternalInput")
    o_d = nc.dram_tensor("o", [HEADS_PER_CORE, L, D], F32, kind="ExternalOutput")

    with tile.TileContext(nc) as tc:
        with (
            tc.tile_pool(name="consts", bufs=1) as consts,
            tc.tile_pool(name="qk", bufs=2) as qk_pool,
            tc.tile_pool(name="vx", bufs=2) as vx_pool,
            tc.tile_pool(name="pt", bufs=6) as pt_pool,
            tc.tile_pool(name="osb", bufs=3) as osb_pool,
            tc.tile_pool(name="small", bufs=3) as small_pool,
            tc.tile_pool(name="outsb", bufs=3) as outsb_pool,
            tc.tile_pool(name="st", bufs=3, space="PSUM") as st_pool,
            tc.tile_pool(name="otp", bufs=1, space="PSUM") as ot_pool,
            tc.tile_pool(name="ops", bufs=1, space="PSUM") as ops_pool,
        ):
            identity = consts.tile([128, 128], F32)
            make_identity(nc, identity)
            # Dummy activation so walrus's ACT table load (~2.7us) runs
            # during the first input DMAs instead of before the first real
            # exp call.
            warm = consts.tile([1, 8], F32)
            nc.vector.memset(warm[:], 0.0)
            nc.scalar.activation(warm[:], warm[:], mybir.ActivationFunctionType.Exp)

            def emit_epilogue(h, q, ot):
                """Transpose O^T_ext back to row-major and normalize by the
                softmax denominators (row 64), then DMA out."""
                osb = osb_pool.tile([D + 1, 512], F32)
                nc.vector.tensor_copy(osb[:], ot[:])
                ops = ops_pool.tile([128, 4, D + 1], F32)
                for qq in range(4):
                    nc.tensor.transpose(
                        ops[:, qq, :],
                        osb[:, qq * 128 : (qq + 1) * 128],
                        identity[0 : D + 1, 0 : D + 1],
                    )
                rcp = small_pool.tile([128, 4], F32)
                nc.vector.reciprocal(rcp[:], ops[:, :, D])
                outsb = outsb_pool.tile([128, 4, D], F32)
                for qq in range(4):
                    nc.vector.tensor_scalar_mul(
                        outsb[:, qq, :], ops[:, qq, 0:D], rcp[:, qq : qq + 1]
                    )
                nc.sync.dma_start(
                    o_d.ap()[h, q * 512 : (q + 1) * 512, :].rearrange(
                        "(a p) d -> p a d", p=128
                    ),
                    outsb[:],
                )

            pending = None  # deferred epilogue of the previous (h, q),
            # emitted after the next tile's first QK group so the PE FIFO
            # isn't head-of-line blocked on its dependencies.
            for h in range(HEADS_PER_CORE):
                qt2 = qk_pool.tile([128, L], MD, tag="qt")
                kt2 = qk_pool.tile([128, L], MD, tag="kt")
                for lo in (0, 1):
                    sl = slice(lo * (L // 2), (lo + 1) * (L // 2))
                    nc.sync.dma_start(qt2[0:64, sl], qt_d.ap()[h][:, sl])
                    nc.sync.dma_start(qt2[64:128, sl], qt_d.ap()[h][:, sl])
                    nc.sync.dma_start(kt2[0:64, sl], kt_d.ap()[h][:, sl])
                    nc.sync.dma_start(kt2[64:128, sl], kt_d.ap()[h][:, sl])
                vx = vx_pool.tile([128, KT, D + 1], MD)
                v_r = v_d.ap()[h].rearrange("(t p) d -> p t d", p=128)
                for c in range(4):
                    nc.sync.dma_start(
                        vx[:, c * 4 : (c + 1) * 4, :], v_r[:, c * 4 : (c + 1) * 4, :]
                    )

                for q in range(QT):
                    ot = ot_pool.tile([D + 1, 512], F32)
                    for g in range(KT // GROUP):
                        st = st_pool.tile([128, 512 * GROUP], F32, tag="st")
                        for i in range(GROUP):
                            kt_idx = GROUP * g + i
                            half = 64 * (kt_idx % 2)
                            nc.tensor.matmul(
                                st[:, i * 512 : (i + 1) * 512],
                                lhsT=kt2[half : half + 64, kt_idx * 128 : (kt_idx + 1) * 128],
                                rhs=qt2[half : half + 64, q * 512 : (q + 1) * 512],
                                start=True,
                                stop=True,
                                tile_position=(half, 0),
                            )
                        pt = pt_pool.tile([128, 512 * GROUP], MD)
                        _act_exp_imm(nc, pt[:], st[:], SCALE)
                        if g == 0 and pending is not None:
                            emit_epilogue(*pending)
                            pending = None
                        for i in range(GROUP):
                            kt_idx = GROUP * g + i
                            nc.tensor.matmul(
                                ot[:, :],
                                lhsT=vx[:, kt_idx, :],
                                rhs=pt[:, i * 512 : (i + 1) * 512],
                                start=(kt_idx == 0),
                                stop=(kt_idx == KT - 1),
                                skip_group_check=True,
                            )
                    pending = (h, q, ot)
            emit_epilogue(*pending)
    _split_sync_waits(nc)
    return nc


def shard_inputs(query, key, value, mm_dtype=MM_DTYPE):
    """Full [B, H, L, D] inputs -> per-core input maps (host-side layout)."""
    np_dt = mybir.dt.np(mm_dtype)
    q = np.asarray(query, dtype=np.float32).reshape(B * H, L, D).astype(np_dt)
    k = np.asarray(key, dtype=np.float32).reshape(B * H, L, D).astype(np_dt)
    v = np.asarray(value, dtype=np.float32).reshape(B * H, L, D).astype(np_dt)
    ones = np.ones((HEADS_PER_CORE, L, 1), np_dt)
    in_maps = []
    for c in range(N_CORES):
        sl = slice(c * HEADS_PER_CORE, (c + 1) * HEADS_PER_CORE)
        in_maps.append(
            {
                "qt": np.ascontiguousarray(q[sl].transpose(0, 2, 1)),
                "kt": np.ascontiguousarray(k[sl].transpose(0, 2, 1)),
                "v": np.ascontiguousarray(np.concatenate([v[sl], ones], axis=-1)),
            }
        )
    return in_maps


def unshard(results):
    """Per-core [heads, L, D] outputs -> full [B, L, H*D]."""
    o = np.concatenate([r["o"] for r in results], axis=0)  # [B*H, L, D]
    o = o.reshape(B, H, L, D).transpose(0, 2, 1, 3).reshape(B, L, H * D)
    return np.ascontiguousarray(o)


_NC_CACHE = {}


def run(query, key, value, trace=False, mm_dtype=MM_DTYPE):
    key_ = mm_dtype
    if key_ not in _NC_CACHE:
        _NC_CACHE[key_] = build_nc(mm_dtype)
    nc = _NC_CACHE[key_]
    in_maps = shard_inputs(query, key, value, mm_dtype)
    res = bass_utils.run_bass_kernel_spmd(
        nc, in_maps, core_ids=list(range(N_CORES)), trace=trace
    )
    return unshard(res.results), res


def kernel(query, key, value, mask=None, to_q=None, to_k=None):
    out, _ = run(query, key, value, trace=False)
    return out


if __name__ == "__main__":
    rng = np.random.default_rng(0)
    q = rng.normal(size=(B, H, L, D)).astype(np.float32)
    k = rng.normal(size=(B, H, L, D)).astype(np.float32)
    v = rng.normal(size=(B, H, L, D)).astype(np.float32)
    out = kernel(q, k, v)
    print("out", out.shape, out.dtype)
